# revision 1
# baseline (speedup 1.0000x reference)
"""Multi-head attention (B=8, S=1024, D=768, H=12, DH=64) on 8 TRN2 NeuronCores.

Strategy: pure data parallelism over batch — core b computes batch element b
end-to-end (no collectives). Per core, activations are kept in transposed
[feature, token] layout so every matmul contracts over the partition dim:

  xT [768,1024]  (host-transposed, bf16)
  qT/kT/vT per head-pair [128,1024] = Wqkv_pair.T @ xT   (PE, k=128, m=128)
  S^T per (pair, tchunk, shalf) [128,1024] = (even|odd) scores, k=64 row-tiled
  P = exp(S^T * 0.125)        (ACT, psum->sbuf bf16)
  O^T[65,512] += [V|1].T @ P  (PE; row 64 = softmax denominator for free)
  normalize: recip(denom) -> DRAM -> partition-broadcast DMA -> DVE multiply
  Y [1024,768] = OT.T @ Wo + bo  (PE k=128; DVE bias-add; natural layout out)

All matmul inputs bf16 (fp32 PSUM accumulation); output f32.
"""

import sys

sys.path.insert(0, "/opt/trn_rl_repo")

import numpy as np
import ml_dtypes

B, S, D = 8, 1024, 768
H = 12
DH = 64
NPAIR = 6  # head pairs
NDC = 6  # 128-wide chunks of D
NTC = 8  # 128-wide chunks of S (key/t side)
NSC = 8  # 128-wide chunks of S (query/s side)

_BF16 = ml_dtypes.bfloat16

_cache = {}


def _build_program():
    import concourse.bass as bass
    import concourse.bacc as bacc
    import concourse.tile as tile
    from concourse import mybir

    F32 = mybir.dt.float32
    BF16 = mybir.dt.bfloat16
    Exp = mybir.ActivationFunctionType.Exp

    nc = bacc.Bacc("TRN2", target_bir_lowering=False, debug=False)

    # ---- DRAM I/O (per core) ----
    xT_d = nc.dram_tensor("xT", [D, S], BF16, kind="ExternalInput")
    wqkv_d = nc.dram_tensor("wqkv", [NPAIR, 128, 3 * D], BF16, kind="ExternalInput")
    wo_d = nc.dram_tensor("wo", [128, NDC * D], BF16, kind="ExternalInput")
    bqkv_d = nc.dram_tensor("bqkv", [128, 18], F32, kind="ExternalInput")
    bo_d = nc.dram_tensor("bo", [1, D], F32, kind="ExternalInput")
    ident_d = nc.dram_tensor("ident", [128, 64], BF16, kind="ExternalInput")
    y_d = nc.dram_tensor("y", [S, D], F32, kind="ExternalOutput")

    denom_d = nc.dram_tensor("denom_scr", [H, S], F32, kind="Internal")

    with tile.TileContext(nc) as tc:
        import contextlib

        ctx = contextlib.ExitStack()
        with ctx:
            const = ctx.enter_context(tc.tile_pool(name="const", bufs=1))
            wpool = ctx.enter_context(tc.tile_pool(name="wpool", bufs=1))
            qkv = ctx.enter_context(tc.tile_pool(name="qkv", bufs=3))
            vn_pool = ctx.enter_context(tc.tile_pool(name="vn", bufs=4))
            ot_pool = ctx.enter_context(tc.tile_pool(name="ot", bufs=1))
            e_pool = ctx.enter_context(tc.tile_pool(name="e", bufs=6))
            r_pool = ctx.enter_context(tc.tile_pool(name="r", bufs=4))
            y_pool = ctx.enter_context(tc.tile_pool(name="ysb", bufs=3))
            ps = ctx.enter_context(tc.tile_pool(name="ps", bufs=1, space="PSUM"))

            # ---- inputs to SBUF; critical-path first (xT + pair-0 weights) ----
            # xT resident tile; first projection's weights go first, then xT
            # chunks (fine-grained deps), then the rest.
            xt_all = wpool.tile([128, NDC * S], BF16, name="xt_all")
            xT = [xt_all[:, dc * S : (dc + 1) * S] for dc in range(NDC)]
            xt_src = xT_d.rearrange("(dc p) s -> p dc s", p=128)

            w_sb = {}
            wqkv_t = {}
            for p in range(NPAIR):
                wqkv_t[p] = wpool.tile([128, 3 * D], BF16, name=f"wqkv{p}")
                for i, wname in enumerate(("q", "k", "v")):
                    w_sb[wname, p] = wqkv_t[p][:, i * D : (i + 1) * D]

            nc.sync.dma_start(w_sb["q", 0], wqkv_d[0, :, 0:D])
            nc.sync.dma_start(xT[0], xt_src[:, 0, :])
            nc.sync.dma_start(xT[1], xt_src[:, 1, :])
            nc.sync.dma_start(w_sb["k", 0], wqkv_d[0, :, D : 2 * D])
            nc.sync.dma_start(xT[2], xt_src[:, 2, :])
            nc.sync.dma_start(xT[3], xt_src[:, 3, :])
            nc.sync.dma_start(w_sb["v", 0], wqkv_d[0, :, 2 * D : 3 * D])
            nc.sync.dma_start(xT[4], xt_src[:, 4, :])
            nc.sync.dma_start(xT[5], xt_src[:, 5, :])
            bqkv = const.tile([128, 18], F32)
            nc.sync.dma_start(bqkv, bqkv_d[:, :])
            ident = const.tile([128, 64], BF16)
            nc.sync.dma_start(ident, ident_d[:, :])
            for p in range(1, NPAIR):
                nc.sync.dma_start(wqkv_t[p], wqkv_d[p, :, :])
            bo_b = const.tile([128, D], F32)
            nc.sync.dma_start(
                bo_b, bass.AP(tensor=bo_d, offset=0, ap=[[0, 128], [1, D]])
            )
            wo_all = wpool.tile([128, NDC * D], BF16, name="wo_all")
            nc.sync.dma_start(wo_all, wo_d[:, :])
            wo_sb = [wo_all[:, dc * D : (dc + 1) * D] for dc in range(NDC)]

            # persistent OT tiles (one per pair, [128, 1024] bf16)
            OT = [ot_pool.tile([128, S], BF16, name=f"OT{p}") for p in range(NPAIR)]

            def projection(p, wname, bias_col):
                """Compute (W_pair.T @ xT + b) -> bf16 [128, 1024] tile."""
                dst = qkv.tile([128, S], BF16, tag=wname, name=f"{wname}T{p}")
                w = w_sb[wname, p]
                for nh in range(2):
                    pst = ps.tile(
                        [128, 512], F32, tag="pp", bufs=2, name=f"pp_{wname}{p}{nh}"
                    )
                    for dc in range(NDC):
                        nc.tensor.matmul(
                            pst,
                            w[:, dc * 128 : (dc + 1) * 128],
                            xT[dc][:, nh * 512 : (nh + 1) * 512],
                            start=(dc == 0),
                            stop=(dc == NDC - 1),
                        )
                    # High priority: these evacs gate the next pair's scores;
                    # they must jump the DVE queue ahead of normalize work.
                    with tc.high_priority(offset=300):
                        nc.vector.tensor_scalar_add(
                            dst[:, nh * 512 : (nh + 1) * 512],
                            pst,
                            bqkv[:, bias_col : bias_col + 1],
                        )
                return dst

            def proj_and_vn(p):
                """Projections + V-natural transpose for pair p."""
                qT = projection(p, "q", 0 * 6 + p)
                kT = projection(p, "k", 1 * 6 + p)
                vT = projection(p, "v", 2 * 6 + p)
                vnat = []
                for h2 in range(2):
                    vps = ps.tile([128, 512], BF16, tag="pp", bufs=2, name=f"vn{p}_{h2}")
                    for tcb in range(NTC):
                        nc.tensor.transpose(
                            vps[:, tcb * 64 : (tcb + 1) * 64],
                            vT[h2 * 64 : (h2 + 1) * 64, tcb * 128 : (tcb + 1) * 128],
                            ident[h2 * 64 : (h2 + 1) * 64, :],
                        )
                    vn = vn_pool.tile([128, NTC * 65], BF16, name=f"vnat{p}_{h2}")
                    vn_r = vn.rearrange("a (b c) -> a b c", c=65)
                    nc.vector.tensor_copy(
                        vn_r[:, :, 0:64], vps.rearrange("a (b c) -> a b c", c=64)
                    )
                    nc.vector.memset(vn_r[:, :, 64:65], 1.0)
                    vnat.append(vn)
                return qT, kT, vnat

            def normalize(p, h2, ou_t, ssl, sh):
                """recip(denom) -> DRAM roundtrip -> partition-bcast -> mult."""
                h = 2 * p + h2
                n = ssl.stop - ssl.start
                rt = r_pool.tile([65, n], F32, tag="rt", bufs=3, name="rt")
                nc.vector.reciprocal(out=rt[64:65, :], in_=ou_t[64:65, ssl])
                nc.sync.dma_start(denom_d[h, ssl], rt[64:65, :])
                rb = r_pool.tile([64, n], F32, tag="rb", bufs=3, name="rb")
                nc.sync.dma_start(
                    rb,
                    bass.AP(
                        tensor=denom_d,
                        offset=h * S + ssl.start,
                        ap=[[0, 64], [1, n]],
                    ),
                )
                nc.vector.tensor_mul(
                    OT[p][h2 * 64 : (h2 + 1) * 64, ssl], ou_t[0:64, ssl], rb
                )

            for p in range(NPAIR):
                qT, kT, vnat = proj_and_vn(p)

                # attention core; s split in halves to fit PSUM
                ou = [
                    r_pool.tile([65, S], F32, tag="ou", bufs=4, name=f"ou{p}_{h2}")
                    for h2 in range(2)
                ]
                for sh in range(2):
                    ssl = slice(sh * 512, (sh + 1) * 512)
                    Ops = [
                        ps.tile([65, 512], F32, tag="o", bufs=2, name=f"o{p}_{sh}_{h2}")
                        for h2 in range(2)
                    ]
                    for tcb in range(NTC):
                        st = ps.tile([128, S], F32, tag="s", bufs=2, name=f"s{p}_{sh}_{tcb}")
                        nc.tensor.matmul(
                            st[:, 0:512],
                            kT[0:64, tcb * 128 : (tcb + 1) * 128],
                            qT[0:64, ssl],
                            start=True,
                            stop=True,
                        )
                        nc.tensor.matmul(
                            st[:, 512:1024],
                            kT[64:128, tcb * 128 : (tcb + 1) * 128],
                            qT[64:128, ssl],
                            start=True,
                            stop=True,
                        )
                        et = e_pool.tile([128, S], BF16, name="expS")
                        nc.scalar.activation(et, st, Exp, scale=0.125)
                        for h2 in range(2):
                            nc.tensor.matmul(
                                Ops[h2][:, :],
                                vnat[h2][:, tcb * 65 : (tcb + 1) * 65],
                                et[:, h2 * 512 : (h2 + 1) * 512],
                                start=(tcb == 0),
                                stop=(tcb == NTC - 1),
                            )
                    # evacuate unnormalized O (frees psum fast)
                    for h2 in range(2):
                        nc.vector.tensor_copy(ou[h2][:, ssl], Ops[h2])
                    if p == NPAIR - 1:
                        # last pair: normalize each s-half as soon as it's
                        # done, so the output projection isn't tail-blocked.
                        for h2 in range(2):
                            normalize(p, h2, ou[h2], ssl, sh)
                if p < NPAIR - 1:
                    # normalize per head, full width, off the critical path
                    for h2 in range(2):
                        normalize(p, h2, ou[h2], slice(0, S), None)

            # ---- output projection: Y[sc] = sum_dc OT[dc][:, sc].T @ Wo[dc] + bo
            for sc in range(NSC):
                yps = ps.tile([128, D], F32, tag="s", bufs=2, name=f"y{sc}")
                for dc in range(NDC):
                    lhsT = OT[dc][:, sc * 128 : (sc + 1) * 128]
                    nc.tensor.matmul(
                        yps[:, 0:512],
                        lhsT,
                        wo_sb[dc][:, 0:512],
                        start=(dc == 0),
                        stop=(dc == NDC - 1),
                    )
                    nc.tensor.matmul(
                        yps[:, 512:768],
                        lhsT,
                        wo_sb[dc][:, 512:768],
                        start=(dc == 0),
                        stop=(dc == NDC - 1),
                    )
                yt = y_pool.tile([128, D], F32, name="yt")
                nc.vector.tensor_add(yt, yps, bo_b)
                nc.sync.dma_start(y_d[sc * 128 : (sc + 1) * 128, :], yt)

    nc.compile()
    return nc


def _prep_inputs(x, Wq, bq, Wk, bk, Wv, bv, Wo, bo):
    """Host-side layout transforms + bf16 casts."""
    x = np.asarray(x)
    # xT per batch: [B, D, S] bf16
    xT = np.ascontiguousarray(x.transpose(0, 2, 1)).astype(_BF16)

    def pack_w(W):
        # W [H, D, DH] -> [NPAIR, 128(d_sub), D(dc*128+m)] where m in 0..127
        # indexes (head-in-pair, e): value[p, d_sub, dc*128+m] = W[2p + m//64, dc*128+d_sub, m%64]
        Wp = np.empty((NPAIR, 128, D), np.float32)
        W = np.asarray(W, np.float32)
        for p in range(NPAIR):
            blk = np.concatenate([W[2 * p], W[2 * p + 1]], axis=1)  # [D, 128]
            # want [d_sub, dc*128+m] = blk[dc*128+d_sub, m]
            Wp[p] = blk.reshape(NDC, 128, 128).transpose(1, 0, 2).reshape(128, D)
        return Wp

    # q|k|v blocks side by side: [NPAIR, 128, 3*768]
    wqkv = np.concatenate([pack_w(Wq), pack_w(Wk), pack_w(Wv)], axis=2).astype(_BF16)

    bqkv = np.empty((128, 18), np.float32)
    for j, b_ in enumerate((bq, bk, bv)):
        b_ = np.asarray(b_, np.float32)
        for p in range(NPAIR):
            bqkv[:, j * 6 + p] = np.concatenate([b_[2 * p], b_[2 * p + 1]])

    Wo = np.asarray(Wo, np.float32)
    # [128(d_sub), NDC*768]: wo[:, dc*768 + j] = Wo[dc*128 + d_sub, j]
    wo = Wo.reshape(NDC, 128, D).transpose(1, 0, 2).reshape(128, NDC * D).astype(_BF16)

    bo_h = np.asarray(bo, np.float32).reshape(1, D)

    ident = np.zeros((128, 64), np.float32)
    ident[0:64] = np.eye(64)
    ident[64:128] = np.eye(64)
    ident = ident.astype(_BF16)

    shared = {
        "wqkv": wqkv,
        "wo": wo,
        "bqkv": bqkv,
        "bo": bo_h,
        "ident": ident,
    }
    return xT, shared


def kernel(x, Wq, bq, Wk, bk, Wv, bv, Wo, bo):
    from concourse.bass_utils import run_bass_kernel_spmd

    if "nc" not in _cache:
        _cache["nc"] = _build_program()
    nc = _cache["nc"]

    xT, shared = _prep_inputs(x, Wq, bq, Wk, bk, Wv, bv, Wo, bo)
    in_maps = [dict(shared, xT=np.ascontiguousarray(xT[b])) for b in range(B)]
    res = run_bass_kernel_spmd(nc, in_maps, core_ids=list(range(B)))
    y = np.stack([res.results[b]["y"] for b in range(B)], axis=0)
    return y.astype(np.float32)



# revision 4
# speedup vs baseline: 1.2673x; 1.2673x over previous
"""Multi-head attention (B=8, S=1024, D=768, H=12, DH=64) on 8 TRN2 NeuronCores.

Data parallel over batch; core b computes batch element b end-to-end.

Per-core design (mixed fp8/bf16):
  q/k path fp8 DoubleRow end-to-end:
    x8T fp8 [128, (c:3, i:2, s:1024)], d = 256c+128i+k
    qT/kT [128(e-pair), S] fp8, zero-padded [128, 2, S]; scores = DR matmul
    K=(64,2) per head at base partition 64*h2 -> st [128(t), 1024] f32 psum
  v path bf16 (error budget):
    v natural [128(t), 2heads*64] = xbT.T @ Wv (bf16), vnat bf16 + ones cols
  P bf16: exp split ACT (Exp activation) / DVE (Schraudolph int16 bitcast)
  AV bf16 flipped: lhsT = P chunk [128(t), 128(s)], rhs = [V|1] -> O [128(s), 65]
  norm: DVE reciprocal (free) + per-partition scale alternating ACT/DVE
  OT: PE transpose; outproj Y = OT.T @ Wo + bo (bf16)

Weights pre-scaled x16 host-side (fp8 subnormal avoidance); exp scale and
Wo absorb the compensation.
"""

import sys

sys.path.insert(0, "/opt/trn_rl_repo")

import numpy as np
import ml_dtypes

B, S, D = 8, 1024, 768
H = 12
DH = 64
NPAIR = 6

_BF16 = ml_dtypes.bfloat16
_F8 = ml_dtypes.float8_e4m3

SW = 16.0  # host weight prescale (q,k,v paths)
SCO = 0.125 / (SW * SW)  # exp scale on raw score psum
LN2 = float(np.log(2.0))
A_SCH = 128.0 * SCO / LN2  # Schraudolph int16 -> bf16
B_SCH = 16256.0 - 7.4

# ---- tuning knobs ----
EXP_ACT_N = 64  # of 96 exp tiles on ACT engine (rest DVE Schraudolph)
QKEVAC = "alt"  # q/k projection evacuation engine: dve | act | alt
NORM = "alt"  # normalize-multiply engine: act | dve | alt
YSB_BUFS = 4

_cache = {}


def _build_program():
    import concourse.bass as bass
    import concourse.bacc as bacc
    import concourse.tile as tile
    from concourse import mybir

    F32 = mybir.dt.float32
    BF16 = mybir.dt.bfloat16
    FP8 = mybir.dt.float8e4
    I16 = mybir.dt.int16
    Exp = mybir.ActivationFunctionType.Exp
    Copy = mybir.ActivationFunctionType.Copy
    Ident = mybir.ActivationFunctionType.Identity
    DR = mybir.MatmulPerfMode.DoubleRow
    MUL = mybir.AluOpType.mult
    ADD = mybir.AluOpType.add

    nc = bacc.Bacc("TRN2", target_bir_lowering=False, debug=False)

    # ---- DRAM I/O (per core) ----
    x8_d = nc.dram_tensor("x8", [128, 6 * S], FP8, kind="ExternalInput")
    xb_d = nc.dram_tensor("xb", [128, 6 * S], BF16, kind="ExternalInput")
    wqk_d = nc.dram_tensor("wqk", [NPAIR, 128, 2 * 768], FP8, kind="ExternalInput")
    wv_d = nc.dram_tensor("wv", [NPAIR, 128, 768], BF16, kind="ExternalInput")
    wo_d = nc.dram_tensor("wo", [128, 6 * D], BF16, kind="ExternalInput")
    bqk_d = nc.dram_tensor("bqk", [128, 12], F32, kind="ExternalInput")
    bvb_d = nc.dram_tensor("bvb", [1, D], F32, kind="ExternalInput")
    bo_d = nc.dram_tensor("bo", [1, D], F32, kind="ExternalInput")
    ident_d = nc.dram_tensor("ident", [128, 128], BF16, kind="ExternalInput")
    y_d = nc.dram_tensor("y", [S, D], F32, kind="ExternalOutput")

    exp_on_act = [
        ((i + 1) * EXP_ACT_N) // 96 - (i * EXP_ACT_N) // 96 > 0 for i in range(96)
    ]

    with tile.TileContext(nc) as tc:
        import contextlib

        ctx = contextlib.ExitStack()
        with ctx:
            const = ctx.enter_context(tc.tile_pool(name="const", bufs=1))
            wpool = ctx.enter_context(tc.tile_pool(name="wpool", bufs=1))
            persist = ctx.enter_context(tc.tile_pool(name="persist", bufs=1))
            et_pool = ctx.enter_context(tc.tile_pool(name="et", bufs=10))
            osb_pool = ctx.enter_context(tc.tile_pool(name="osb", bufs=4))
            rcp_pool = ctx.enter_context(tc.tile_pool(name="rcp", bufs=8))
            ysb_pool = ctx.enter_context(tc.tile_pool(name="ysb", bufs=YSB_BUFS))
            ps = ctx.enter_context(tc.tile_pool(name="ps", bufs=1, space="PSUM"))

            # ---- load inputs; critical path (pair-0 weights, x) first ----
            wqk_t = {
                p: wpool.tile([128, 2, 3, 2, 128], FP8, name=f"wqk{p}")
                for p in range(NPAIR)
            }
            wv_t = {
                p: wpool.tile([128, 6, 128], BF16, name=f"wv{p}")
                for p in range(NPAIR)
            }

            nc.sync.dma_start(
                wqk_t[0].rearrange("p a b c d -> p (a b c d)"), wqk_d[0, :, :]
            )
            x8 = wpool.tile([128, 3, 2, S], FP8, name="x8")
            nc.sync.dma_start(x8.rearrange("p a b s -> p (a b s)"), x8_d[:, :])
            xb = wpool.tile([128, 6, S], BF16, name="xb")
            nc.sync.dma_start(xb.rearrange("p a s -> p (a s)"), xb_d[:, :])
            nc.sync.dma_start(wv_t[0].rearrange("p a b -> p (a b)"), wv_d[0, :, :])
            bqk = const.tile([128, 12], F32)
            nc.sync.dma_start(bqk, bqk_d[:, :])
            bvb = const.tile([128, D], F32)
            nc.sync.dma_start(
                bvb, bass.AP(tensor=bvb_d, offset=0, ap=[[0, 128], [1, D]])
            )
            for p in range(1, NPAIR):
                nc.sync.dma_start(
                    wqk_t[p].rearrange("p a b c d -> p (a b c d)"), wqk_d[p, :, :]
                )
                nc.sync.dma_start(
                    wv_t[p].rearrange("p a b -> p (a b)"), wv_d[p, :, :]
                )
            ident = const.tile([128, 128], BF16)
            nc.sync.dma_start(ident, ident_d[:, :])
            bo_b = const.tile([128, D], F32)
            nc.sync.dma_start(
                bo_b, bass.AP(tensor=bo_d, offset=0, ap=[[0, 128], [1, D]])
            )
            wo_all = wpool.tile([128, 6, D], BF16, name="wo_all")
            nc.sync.dma_start(wo_all.rearrange("p a b -> p (a b)"), wo_d[:, :])

            # ---- persistent ping-pong tiles ----
            q8b = [persist.tile([128, 2, S], FP8, name=f"q8_{j}") for j in range(2)]
            k8b = [persist.tile([128, 2, S], FP8, name=f"k8_{j}") for j in range(2)]
            vnb = [
                persist.tile([128, 4, 2, 130], BF16, name=f"vn_{j}")
                for j in range(2)
            ]
            for j in range(2):
                nc.vector.memset(q8b[j][:, 1, :], 0.0)
                nc.vector.memset(k8b[j][:, 1, :], 0.0)
                nc.vector.memset(vnb[j][:, :, :, 64:65], 1.0)
                nc.vector.memset(vnb[j][:, :, :, 129:130], 1.0)

            OT_sb = [
                persist.tile([128, S], BF16, name=f"OT{p}") for p in range(NPAIR)
            ]

            exp_i = [0]
            norm_i = [0]

            def proj_block(p):
                """PE projections for pair p: q/k (fp8 DR) + v natural (bf16)."""
                q8, k8, vn = q8b[p % 2], k8b[p % 2], vnb[p % 2]
                for j, dst in ((0, q8), (1, k8)):
                    for sh in range(2):
                        pp = ps.tile(
                            [128, 512], F32, tag="pp", bufs=2, name=f"pp{p}{j}{sh}"
                        )
                        for c in range(3):
                            nc.tensor.matmul(
                                pp,
                                wqk_t[p][:, j, c],
                                x8[:, c, :, sh * 512 : (sh + 1) * 512],
                                start=(c == 0),
                                stop=(c == 2),
                                perf_mode=DR,
                            )
                        out = dst[:, 0, sh * 512 : (sh + 1) * 512]
                        col = j * 6 + p
                        eng = QKEVAC if QKEVAC != "alt" else ("dve", "act")[sh]
                        with tc.high_priority(offset=300):
                            if eng == "act":
                                nc.scalar.activation(
                                    out, pp, Ident, bias=bqk[:, col : col + 1]
                                )
                            else:
                                nc.vector.tensor_scalar_add(
                                    out, pp, bqk[:, col : col + 1]
                                )

                for tcc in range(8):
                    vp = ps.tile([128, 128], F32, tag="pp", bufs=2, name=f"vp{p}{tcc}")
                    for dc in range(6):
                        nc.tensor.matmul(
                            vp,
                            xb[:, dc, tcc * 128 : (tcc + 1) * 128],
                            wv_t[p][:, dc],
                            start=(dc == 0),
                            stop=(dc == 5),
                        )
                    c4, i2 = divmod(tcc, 2)
                    dst = vn[:, c4, i2, 0:130].rearrange("p (h e) -> p h e", h=2)[
                        :, :, 0:64
                    ]
                    src = vp.rearrange("p (h e) -> p h e", h=2)
                    bvs = bvb[:, p * 128 : (p + 1) * 128].rearrange(
                        "p (h e) -> p h e", h=2
                    )
                    nc.vector.tensor_add(dst, src, bvs)

            def scores_exp_block(p, sh):
                """Scores (DR fp8) + exp (ACT/DVE split) -> et tiles (bf16)."""
                q8, k8 = q8b[p % 2], k8b[p % 2]
                ets = [
                    et_pool.tile([128, 2, S], BF16, tag="et", name=f"et{p}{sh}{c}")
                    for c in range(4)
                ]
                for tcb in range(8):
                    st = ps.tile(
                        [128, S], F32, tag="st", bufs=2, name=f"st{p}{sh}{tcb}"
                    )
                    for h2 in range(2):
                        nc.tensor.matmul(
                            st[:, h2 * 512 : (h2 + 1) * 512],
                            k8[h2 * 64 : h2 * 64 + 64, :, tcb * 128 : (tcb + 1) * 128],
                            q8[h2 * 64 : h2 * 64 + 64, :, sh * 512 : (sh + 1) * 512],
                            start=True,
                            stop=True,
                            perf_mode=DR,
                            tile_position=(h2 * 64, 0),
                        )
                    c4, i2 = divmod(tcb, 2)
                    dst = ets[c4][:, i2, :]
                    if exp_on_act[exp_i[0]]:
                        nc.scalar.activation(dst, st, Exp, scale=SCO)
                    else:
                        nc.vector.tensor_scalar(
                            dst.bitcast(I16), st, A_SCH, B_SCH, MUL, ADD
                        )
                    exp_i[0] += 1
                return ets

            def av_block(p, sh, ets, osb):
                """AV (bf16 flipped) + normalize for s-half sh."""
                vn = vnb[p % 2]
                for h2 in range(2):
                    for sc4 in range(4):
                        sc = sh * 4 + sc4
                        O = ps.tile(
                            [128, 65], F32, tag="o", bufs=2, name=f"o{p}{h2}{sc}"
                        )
                        off = h2 * 512 + sc4 * 128
                        for tcb in range(8):
                            c4, i2 = divmod(tcb, 2)
                            nc.tensor.matmul(
                                O,
                                ets[c4][:, i2, off : off + 128],
                                vn[:, c4, i2, h2 * 65 : (h2 + 1) * 65],
                                start=(tcb == 0),
                                stop=(tcb == 7),
                            )
                        rcp = rcp_pool.tile(
                            [128, 1], F32, tag="rcp", name=f"r{p}{h2}{sc}"
                        )
                        nc.vector.reciprocal(rcp, O[:, 64:65])
                        out = osb[h2][:, sc * 64 : (sc + 1) * 64]
                        eng = NORM if NORM != "alt" else ("dve", "act")[norm_i[0] % 2]
                        norm_i[0] += 1
                        if eng == "act":
                            nc.scalar.activation(out, O[:, 0:64], Copy, scale=rcp)
                        else:
                            nc.vector.tensor_scalar_mul(out, O[:, 0:64], rcp)

            def transpose_block(p, osb):
                OT_ps = ps.tile([128, S], BF16, tag="o", bufs=2, name=f"otp{p}")
                for h2 in range(2):
                    for sc in range(8):
                        nc.tensor.transpose(
                            OT_ps[h2 * 64 : h2 * 64 + 64, sc * 128 : (sc + 1) * 128],
                            osb[h2][:, sc * 64 : (sc + 1) * 64],
                            ident,
                            tile_position=(0, h2 * 64),
                        )
                nc.vector.tensor_copy(OT_sb[p], OT_ps)

            # ---- software-pipelined pair loop ----
            proj_block(0)
            for p in range(NPAIR):
                osb = {
                    h2: osb_pool.tile(
                        [128, 512], BF16, tag="osb", name=f"osb{p}{h2}"
                    )
                    for h2 in range(2)
                }
                ets0 = scores_exp_block(p, 0)
                ets1 = scores_exp_block(p, 1)
                av_block(p, 0, ets0, osb)
                if p + 1 < NPAIR:
                    proj_block(p + 1)  # PE fills the exp(sh1) wait
                av_block(p, 1, ets1, osb)
                transpose_block(p, osb)

            # ---- output projection ----
            for sc in range(8):
                Y = ps.tile([128, D], F32, tag="st", bufs=2, name=f"y{sc}")
                for dc in range(6):
                    lhsT = OT_sb[dc][:, sc * 128 : (sc + 1) * 128]
                    nc.tensor.matmul(
                        Y[:, 0:512],
                        lhsT,
                        wo_all[:, dc, 0:512],
                        start=(dc == 0),
                        stop=(dc == 5),
                    )
                    nc.tensor.matmul(
                        Y[:, 512:768],
                        lhsT,
                        wo_all[:, dc, 512:768],
                        start=(dc == 0),
                        stop=(dc == 5),
                    )
                ysb = ysb_pool.tile([128, D], F32, tag="ysb", name=f"ysb{sc}")
                nc.vector.tensor_add(ysb, Y, bo_b)
                nc.sync.dma_start(y_d[sc * 128 : (sc + 1) * 128, :], ysb)

    nc.compile()
    return nc


def _prep_inputs(x, Wq, bq, Wk, bk, Wv, bv, Wo, bo):
    """Host-side layout transforms + fp8/bf16 casts."""
    x = np.asarray(x)
    xT = np.ascontiguousarray(x.transpose(0, 2, 1))  # [B, D, S]
    xch = xT.reshape(B, 6, 128, S).transpose(0, 2, 1, 3).reshape(B, 128, 6 * S)
    x8 = xch.astype(_F8)
    xbf = xch.astype(_BF16)

    def pack_pair_dr(Wa, Wb):
        # [D,64]x2 -> [128(k), 3(c), 2(i), 128(m)], row d = 256c+128i+k
        blk = np.concatenate([Wa, Wb], axis=1).astype(np.float32) * SW  # [768,128]
        return blk.reshape(3, 2, 128, 128).transpose(2, 0, 1, 3).reshape(128, 768)

    def pack_pair_dc(Wa, Wb):
        # [D,64]x2 -> [128(k), 6(dc), 128(m)], row d = 128*dc + k
        blk = np.concatenate([Wa, Wb], axis=1).astype(np.float32) * SW
        return blk.reshape(6, 128, 128).transpose(1, 0, 2).reshape(128, 768)

    Wq = np.asarray(Wq, np.float32)
    Wk = np.asarray(Wk, np.float32)
    Wv = np.asarray(Wv, np.float32)
    wqk = np.empty((NPAIR, 128, 2 * 768), np.float32)
    wv = np.empty((NPAIR, 128, 768), np.float32)
    for p in range(NPAIR):
        wqk[p, :, 0:768] = pack_pair_dr(Wq[2 * p], Wq[2 * p + 1])
        wqk[p, :, 768:1536] = pack_pair_dr(Wk[2 * p], Wk[2 * p + 1])
        wv[p] = pack_pair_dc(Wv[2 * p], Wv[2 * p + 1])
    wqk = wqk.astype(_F8)
    wv = wv.astype(_BF16)

    bqk = np.empty((128, 12), np.float32)
    for j, b_ in enumerate((bq, bk)):
        b_ = np.asarray(b_, np.float32) * SW
        for p in range(NPAIR):
            bqk[:, j * 6 + p] = np.concatenate([b_[2 * p], b_[2 * p + 1]])
    bvb = (np.asarray(bv, np.float32) * SW).reshape(1, D)

    Wo = np.asarray(Wo, np.float32) / SW
    wo = Wo.reshape(6, 128, D).transpose(1, 0, 2).reshape(128, 6 * D).astype(_BF16)
    bo_h = np.asarray(bo, np.float32).reshape(1, D)

    ident = np.eye(128, dtype=np.float32).astype(_BF16)

    shared = {
        "wqk": wqk,
        "wv": wv,
        "wo": wo,
        "bqk": bqk,
        "bvb": bvb,
        "bo": bo_h,
        "ident": ident,
    }
    return x8, xbf, shared


def kernel(x, Wq, bq, Wk, bk, Wv, bv, Wo, bo):
    from concourse.bass_utils import run_bass_kernel_spmd

    if "nc" not in _cache:
        _cache["nc"] = _build_program()
    nc = _cache["nc"]

    x8, xbf, shared = _prep_inputs(x, Wq, bq, Wk, bk, Wv, bv, Wo, bo)
    in_maps = [
        dict(
            shared,
            x8=np.ascontiguousarray(x8[b]),
            xb=np.ascontiguousarray(xbf[b]),
        )
        for b in range(B)
    ]
    res = run_bass_kernel_spmd(nc, in_maps, core_ids=list(range(B)))
    y = np.stack([res.results[b]["y"] for b in range(B)], axis=0)
    return y.astype(np.float32)


# revision 8
# speedup vs baseline: 1.2995x; 1.0254x over previous
"""Multi-head attention (B=8, S=1024, D=768, H=12, DH=64) on 8 TRN2 NeuronCores.

Data parallel over batch; core b computes batch element b end-to-end.

Per-core design (mixed fp8/bf16):
  q/k path fp8 DoubleRow end-to-end:
    x8T fp8 [128, (c:3, i:2, s:1024)], d = 256c+128i+k
    qT/kT [128(e-pair), S] fp8, zero-padded [128, 2, S]; scores = DR matmul
    K=(64,2) per head at base partition 64*h2 -> st [128(t), 1024] f32 psum
  v path bf16 (error budget):
    v natural [128(t), 2heads*64] = xbT.T @ Wv (bf16), vnat bf16 + ones cols
  P bf16: exp split ACT (Exp activation) / DVE (Schraudolph int16 bitcast)
  AV bf16 flipped: lhsT = P chunk [128(t), 128(s)], rhs = [V|1] -> O [128(s), 65]
  norm: DVE reciprocal (free) + per-partition scale alternating ACT/DVE
  OT: PE transpose; outproj Y = OT.T @ Wo + bo (bf16)

Weights pre-scaled x16 host-side (fp8 subnormal avoidance); exp scale and
Wo absorb the compensation.
"""

import sys

sys.path.insert(0, "/opt/trn_rl_repo")

import numpy as np
import ml_dtypes

B, S, D = 8, 1024, 768
H = 12
DH = 64
NPAIR = 6

_BF16 = ml_dtypes.bfloat16
_F8 = ml_dtypes.float8_e4m3

SW = 16.0  # host weight prescale (q,k,v paths)
SCO = 0.125 / (SW * SW)  # exp scale on raw score psum
LN2 = float(np.log(2.0))
A_SCH = 128.0 * SCO / LN2  # Schraudolph int16 -> bf16
B_SCH = 16256.0 - 7.4

# ---- tuning knobs ----
EXP_ACT_N = 64  # of 96 exp tiles on ACT engine (rest DVE Schraudolph)
QKEVAC = "alt"  # q/k projection evacuation engine: dve | act | alt
NORM = "alt"  # normalize-multiply engine: act | dve | alt
YSB_BUFS = 4

_cache = {}


def _build_program():
    import concourse.bass as bass
    import concourse.bacc as bacc
    import concourse.tile as tile
    from concourse import mybir

    F32 = mybir.dt.float32
    BF16 = mybir.dt.bfloat16
    FP8 = mybir.dt.float8e4
    I16 = mybir.dt.int16
    Exp = mybir.ActivationFunctionType.Exp
    Copy = mybir.ActivationFunctionType.Copy
    Ident = mybir.ActivationFunctionType.Identity
    DR = mybir.MatmulPerfMode.DoubleRow
    MUL = mybir.AluOpType.mult
    ADD = mybir.AluOpType.add

    nc = bacc.Bacc("TRN2", target_bir_lowering=False, debug=False)

    # ---- DRAM I/O (per core) ----
    x8_d = nc.dram_tensor("x8", [128, 6 * S], FP8, kind="ExternalInput")
    xb_d = nc.dram_tensor("xb", [128, 6 * S], BF16, kind="ExternalInput")
    wqk_d = nc.dram_tensor("wqk", [NPAIR, 128, 2 * 768], FP8, kind="ExternalInput")
    wv_d = nc.dram_tensor("wv", [NPAIR, 128, 768], BF16, kind="ExternalInput")
    wo_d = nc.dram_tensor("wo", [128, 6 * D], BF16, kind="ExternalInput")
    bqk_d = nc.dram_tensor("bqk", [128, 12], F32, kind="ExternalInput")
    bvb_d = nc.dram_tensor("bvb", [1, D], F32, kind="ExternalInput")
    bo_d = nc.dram_tensor("bo", [1, D], F32, kind="ExternalInput")
    ident_d = nc.dram_tensor("ident", [128, 128], BF16, kind="ExternalInput")
    y_d = nc.dram_tensor("y", [S, D], F32, kind="ExternalOutput")

    exp_on_act = [
        ((i + 1) * EXP_ACT_N) // 96 - (i * EXP_ACT_N) // 96 > 0 for i in range(96)
    ]

    with tile.TileContext(nc) as tc:
        import contextlib

        ctx = contextlib.ExitStack()
        with ctx:
            const = ctx.enter_context(tc.tile_pool(name="const", bufs=1))
            wpool = ctx.enter_context(tc.tile_pool(name="wpool", bufs=1))
            persist = ctx.enter_context(tc.tile_pool(name="persist", bufs=1))
            et_pool = ctx.enter_context(tc.tile_pool(name="et", bufs=14))
            osb_pool = ctx.enter_context(tc.tile_pool(name="osb", bufs=4))
            rcp_pool = ctx.enter_context(tc.tile_pool(name="rcp", bufs=8))
            ysb_pool = ctx.enter_context(tc.tile_pool(name="ysb", bufs=YSB_BUFS))
            ps = ctx.enter_context(tc.tile_pool(name="ps", bufs=1, space="PSUM"))

            # ---- load inputs; critical path (pair-0 weights, x) first ----
            wqk_t = {
                p: wpool.tile([128, 2, 3, 2, 128], FP8, name=f"wqk{p}")
                for p in range(NPAIR)
            }
            wv_t = {
                p: wpool.tile([128, 6, 128], BF16, name=f"wv{p}")
                for p in range(NPAIR)
            }

            nc.sync.dma_start(
                wqk_t[0].rearrange("p a b c d -> p (a b c d)"), wqk_d[0, :, :]
            )
            x8 = wpool.tile([128, 3, 2, S], FP8, name="x8")
            nc.sync.dma_start(x8.rearrange("p a b s -> p (a b s)"), x8_d[:, :])
            bqk = const.tile([128, 12], F32)
            nc.sync.dma_start(bqk, bqk_d[:, :])
            xb = wpool.tile([128, 6, S], BF16, name="xb")
            nc.sync.dma_start(xb.rearrange("p a s -> p (a s)"), xb_d[:, :])
            nc.sync.dma_start(wv_t[0].rearrange("p a b -> p (a b)"), wv_d[0, :, :])
            bvb = const.tile([128, D], F32)
            nc.sync.dma_start(
                bvb, bass.AP(tensor=bvb_d, offset=0, ap=[[0, 128], [1, D]])
            )
            for p in range(1, NPAIR):
                nc.sync.dma_start(
                    wqk_t[p].rearrange("p a b c d -> p (a b c d)"), wqk_d[p, :, :]
                )
                nc.sync.dma_start(
                    wv_t[p].rearrange("p a b -> p (a b)"), wv_d[p, :, :]
                )
            ident = const.tile([128, 128], BF16)
            nc.sync.dma_start(ident, ident_d[:, :])
            bo_b = const.tile([128, D], F32)
            nc.sync.dma_start(
                bo_b, bass.AP(tensor=bo_d, offset=0, ap=[[0, 128], [1, D]])
            )
            wo_all = wpool.tile([128, 6, D], BF16, name="wo_all")
            nc.sync.dma_start(wo_all.rearrange("p a b -> p (a b)"), wo_d[:, :])

            # ---- persistent ping-pong tiles ----
            q8b = [persist.tile([128, 2, S], FP8, name=f"q8_{j}") for j in range(2)]
            k8b = [persist.tile([128, 2, S], FP8, name=f"k8_{j}") for j in range(2)]
            vnb = [
                persist.tile([128, 4, 2, 130], BF16, name=f"vn_{j}")
                for j in range(3)
            ]
            for j in range(2):
                nc.gpsimd.memset(q8b[j][:, 1, :], 0.0)
                nc.gpsimd.memset(k8b[j][:, 1, :], 0.0)
            for j in range(3):
                nc.gpsimd.memset(vnb[j][:, :, :, 64:65], 1.0)
                nc.gpsimd.memset(vnb[j][:, :, :, 129:130], 1.0)

            OT_sb = [
                persist.tile([128, S], BF16, name=f"OT{p}") for p in range(NPAIR)
            ]

            exp_i = [0]
            norm_i = [0]

            def proj_block(p):
                """PE projections for pair p: q/k (fp8 DR) + v natural (bf16)."""
                q8, k8, vn = q8b[p % 2], k8b[p % 2], vnb[p % 3]
                for sh in range(2):
                    for j, dst in ((0, q8), (1, k8)):
                        pp = ps.tile(
                            [128, 512], F32, tag="pp", bufs=2, name=f"pp{p}{j}{sh}"
                        )
                        for c in range(3):
                            nc.tensor.matmul(
                                pp,
                                wqk_t[p][:, j, c],
                                x8[:, c, :, sh * 512 : (sh + 1) * 512],
                                start=(c == 0),
                                stop=(c == 2),
                                perf_mode=DR,
                            )
                        out = dst[:, 0, sh * 512 : (sh + 1) * 512]
                        col = j * 6 + p
                        eng = QKEVAC if QKEVAC != "alt" else ("dve", "act")[sh]
                        with tc.high_priority(offset=300):
                            if eng == "act":
                                nc.scalar.activation(
                                    out, pp, Ident, bias=bqk[:, col : col + 1]
                                )
                            else:
                                nc.vector.tensor_scalar_add(
                                    out, pp, bqk[:, col : col + 1]
                                )

                for tcc in range(8):
                    vp = ps.tile([128, 128], F32, tag="pp", bufs=2, name=f"vp{p}{tcc}")
                    for dc in range(6):
                        nc.tensor.matmul(
                            vp,
                            xb[:, dc, tcc * 128 : (tcc + 1) * 128],
                            wv_t[p][:, dc],
                            start=(dc == 0),
                            stop=(dc == 5),
                        )
                    c4, i2 = divmod(tcc, 2)
                    dst = vn[:, c4, i2, 0:130].rearrange("p (h e) -> p h e", h=2)[
                        :, :, 0:64
                    ]
                    src = vp.rearrange("p (h e) -> p h e", h=2)
                    bvs = bvb[:, p * 128 : (p + 1) * 128].rearrange(
                        "p (h e) -> p h e", h=2
                    )
                    nc.vector.tensor_add(dst, src, bvs)

            def scores_exp_block(p, sh):
                """Scores (DR fp8) + exp (ACT/DVE split) -> et tiles (bf16)."""
                q8, k8 = q8b[p % 2], k8b[p % 2]
                ets = [
                    et_pool.tile([128, 2, S], BF16, tag="et", name=f"et{p}{sh}{c}")
                    for c in range(4)
                ]
                for tcb in range(8):
                    st = ps.tile(
                        [128, S], F32, tag="st", bufs=2, name=f"st{p}{sh}{tcb}"
                    )
                    for h2 in range(2):
                        nc.tensor.matmul(
                            st[:, h2 * 512 : (h2 + 1) * 512],
                            k8[h2 * 64 : h2 * 64 + 64, :, tcb * 128 : (tcb + 1) * 128],
                            q8[h2 * 64 : h2 * 64 + 64, :, sh * 512 : (sh + 1) * 512],
                            start=True,
                            stop=True,
                            perf_mode=DR,
                            tile_position=(h2 * 64, 0),
                        )
                    c4, i2 = divmod(tcb, 2)
                    dst = ets[c4][:, i2, :]
                    if exp_on_act[exp_i[0]]:
                        nc.scalar.activation(dst, st, Exp, scale=SCO)
                    else:
                        nc.vector.tensor_scalar(
                            dst.bitcast(I16), st, A_SCH, B_SCH, MUL, ADD
                        )
                    exp_i[0] += 1
                return ets

            def av_block(p, sh, ets, osb):
                """AV (bf16 flipped) + normalize for s-half sh."""
                vn = vnb[p % 3]
                for h2 in range(2):
                    for sc4 in range(4):
                        sc = sh * 4 + sc4
                        O = ps.tile(
                            [128, 65], F32, tag="o", bufs=2, name=f"o{p}{h2}{sc}"
                        )
                        off = h2 * 512 + sc4 * 128
                        for tcb in range(8):
                            c4, i2 = divmod(tcb, 2)
                            nc.tensor.matmul(
                                O,
                                ets[c4][:, i2, off : off + 128],
                                vn[:, c4, i2, h2 * 65 : (h2 + 1) * 65],
                                start=(tcb == 0),
                                stop=(tcb == 7),
                            )
                        rcp = rcp_pool.tile(
                            [128, 1], F32, tag="rcp", name=f"r{p}{h2}{sc}"
                        )
                        nc.vector.reciprocal(rcp, O[:, 64:65])
                        out = osb[h2][:, sc * 64 : (sc + 1) * 64]
                        eng = NORM if NORM != "alt" else ("dve", "act")[norm_i[0] % 2]
                        norm_i[0] += 1
                        if eng == "act":
                            nc.scalar.activation(out, O[:, 0:64], Copy, scale=rcp)
                        else:
                            nc.vector.tensor_scalar_mul(out, O[:, 0:64], rcp)

            def transpose_block(p, osb):
                OT_ps = ps.tile([128, S], BF16, tag="o", bufs=2, name=f"otp{p}")
                for h2 in range(2):
                    for sc in range(8):
                        nc.tensor.transpose(
                            OT_ps[h2 * 64 : h2 * 64 + 64, sc * 128 : (sc + 1) * 128],
                            osb[h2][:, sc * 64 : (sc + 1) * 64],
                            ident,
                            tile_position=(0, h2 * 64),
                        )
                nc.vector.tensor_copy(OT_sb[p], OT_ps)

            # ---- half-pair-skewed pipeline: AV(p, sh) lags scores by one
            # s-half so exp (ACT/DVE) never blocks the PE stream.
            osbs = {}
            etss = {}
            proj_block(0)
            etss[0, 0] = scores_exp_block(0, 0)
            for p in range(NPAIR):
                osbs[p] = {
                    h2: osb_pool.tile(
                        [128, 512], BF16, tag="osb", name=f"osb{p}{h2}"
                    )
                    for h2 in range(2)
                }
                if p > 0:
                    etss[p, 0] = scores_exp_block(p, 0)
                    av_block(p - 1, 1, etss[p - 1, 1], osbs[p - 1])
                    transpose_block(p - 1, osbs.pop(p - 1))
                etss[p, 1] = scores_exp_block(p, 1)
                av_block(p, 0, etss.pop((p, 0)), osbs[p])
                if p + 1 < NPAIR:
                    proj_block(p + 1)
            av_block(NPAIR - 1, 1, etss[NPAIR - 1, 1], osbs[NPAIR - 1])
            transpose_block(NPAIR - 1, osbs[NPAIR - 1])

            # ---- output projection ----
            for sc in range(8):
                Y = ps.tile([128, D], F32, tag="st", bufs=2, name=f"y{sc}")
                for dc in range(6):
                    lhsT = OT_sb[dc][:, sc * 128 : (sc + 1) * 128]
                    nc.tensor.matmul(
                        Y[:, 0:512],
                        lhsT,
                        wo_all[:, dc, 0:512],
                        start=(dc == 0),
                        stop=(dc == 5),
                    )
                    nc.tensor.matmul(
                        Y[:, 512:768],
                        lhsT,
                        wo_all[:, dc, 512:768],
                        start=(dc == 0),
                        stop=(dc == 5),
                    )
                ysb = ysb_pool.tile([128, D], F32, tag="ysb", name=f"ysb{sc}")
                nc.vector.tensor_add(ysb, Y, bo_b)
                nc.sync.dma_start(y_d[sc * 128 : (sc + 1) * 128, :], ysb)

    nc.compile()
    return nc


def _prep_inputs(x, Wq, bq, Wk, bk, Wv, bv, Wo, bo):
    """Host-side layout transforms + fp8/bf16 casts."""
    x = np.asarray(x)
    xT = np.ascontiguousarray(x.transpose(0, 2, 1))  # [B, D, S]
    xch = xT.reshape(B, 6, 128, S).transpose(0, 2, 1, 3).reshape(B, 128, 6 * S)
    x8 = xch.astype(_F8)
    xbf = xch.astype(_BF16)

    def pack_pair_dr(Wa, Wb):
        # [D,64]x2 -> [128(k), 3(c), 2(i), 128(m)], row d = 256c+128i+k
        blk = np.concatenate([Wa, Wb], axis=1).astype(np.float32) * SW  # [768,128]
        return blk.reshape(3, 2, 128, 128).transpose(2, 0, 1, 3).reshape(128, 768)

    def pack_pair_dc(Wa, Wb):
        # [D,64]x2 -> [128(k), 6(dc), 128(m)], row d = 128*dc + k
        blk = np.concatenate([Wa, Wb], axis=1).astype(np.float32) * SW
        return blk.reshape(6, 128, 128).transpose(1, 0, 2).reshape(128, 768)

    Wq = np.asarray(Wq, np.float32)
    Wk = np.asarray(Wk, np.float32)
    Wv = np.asarray(Wv, np.float32)
    wqk = np.empty((NPAIR, 128, 2 * 768), np.float32)
    wv = np.empty((NPAIR, 128, 768), np.float32)
    for p in range(NPAIR):
        wqk[p, :, 0:768] = pack_pair_dr(Wq[2 * p], Wq[2 * p + 1])
        wqk[p, :, 768:1536] = pack_pair_dr(Wk[2 * p], Wk[2 * p + 1])
        wv[p] = pack_pair_dc(Wv[2 * p], Wv[2 * p + 1])
    wqk = wqk.astype(_F8)
    wv = wv.astype(_BF16)

    bqk = np.empty((128, 12), np.float32)
    for j, b_ in enumerate((bq, bk)):
        b_ = np.asarray(b_, np.float32) * SW
        for p in range(NPAIR):
            bqk[:, j * 6 + p] = np.concatenate([b_[2 * p], b_[2 * p + 1]])
    bvb = (np.asarray(bv, np.float32) * SW).reshape(1, D)

    Wo = np.asarray(Wo, np.float32) / SW
    wo = Wo.reshape(6, 128, D).transpose(1, 0, 2).reshape(128, 6 * D).astype(_BF16)
    bo_h = np.asarray(bo, np.float32).reshape(1, D)

    ident = np.eye(128, dtype=np.float32).astype(_BF16)

    shared = {
        "wqk": wqk,
        "wv": wv,
        "wo": wo,
        "bqk": bqk,
        "bvb": bvb,
        "bo": bo_h,
        "ident": ident,
    }
    return x8, xbf, shared


def kernel(x, Wq, bq, Wk, bk, Wv, bv, Wo, bo):
    from concourse.bass_utils import run_bass_kernel_spmd

    if "nc" not in _cache:
        _cache["nc"] = _build_program()
    nc = _cache["nc"]

    x8, xbf, shared = _prep_inputs(x, Wq, bq, Wk, bk, Wv, bv, Wo, bo)
    in_maps = [
        dict(
            shared,
            x8=np.ascontiguousarray(x8[b]),
            xb=np.ascontiguousarray(xbf[b]),
        )
        for b in range(B)
    ]
    res = run_bass_kernel_spmd(nc, in_maps, core_ids=list(range(B)))
    y = np.stack([res.results[b]["y"] for b in range(B)], axis=0)
    return y.astype(np.float32)
